# revision 23
# baseline (speedup 1.0000x reference)
"""Multi-head attention (B=2, S=2048, H=2048, 16 heads, d=128) on 8 TRN2
NeuronCores.

Sharding: 2-way batch x 4-way head-group tensor parallel. Core j handles
batch j//4 and heads 4*(j%4)..4*(j%4)+3 (a 512-wide slice of the qkv
projection output dim / o_proj input dim). Each core returns a partial
o_proj output [S, H]; the host sums the 4 partials per batch and adds bo.

On-device compute per core (fp16 matmul operands, fp32 PSUM accumulation),
arranged as one interleaved PE stream so the tensor engine never waits on
the ACT-bound attention inner loop:

  P1: v = xT.T @ wvT (+bias) and head 0's qT/kT projections, seq-blocked.
  P2: for h in 0..2: attention(h) stages with head h+1's qT/kT projection
      matmuls as per-stage PE filler (x re-streamed from HBM per head).
  P3: attention(h=3) with o_proj matmuls of completed q-blocks as filler.
  P4: leftover o_proj + output staging.

Attention per (head, q-block), scoresT layout [k, q]:
  scoresT = kT_h.T @ qT_h -> exp on ScalarE -> expT (fp16)
  VectorE pair-adds exp chunks; sums += ones128.T @ pair  (half-cost rowsum)
  ctxT += v_chunk.T @ expT ; ctxT_norm = ctxT * approx_recip(sums)
"""
import sys

if "/opt/trn_rl_repo" not in sys.path:
    sys.path.insert(0, "/opt/trn_rl_repo")

import numpy as np

HIDDEN = 2048
HEADS = 16
HEAD_DIM = 128
BATCH = 2
SEQ = 2048

N_CORES = 8
GROUPS = 4               # head groups (cores per batch)
GDIM = HIDDEN // GROUPS  # 512 dims per core
GHEADS = GDIM // HEAD_DIM  # 4 heads per core
KC = HIDDEN // 128       # 16 contraction chunks
SB = 4                   # seq blocks of 512
QB = SEQ // 512          # 4 q-blocks in attention
MT = SEQ // 128          # 16 seq tiles of 128
NST = QB * KC // 2       # 32 attention stages per head

_CACHE = {}


def _build():
    import concourse.bacc as bacc
    import concourse.bass as bass
    import concourse.mybir as mybir
    import concourse.tile as tile

    fp16 = mybir.dt.float16
    fp32 = mybir.dt.float32
    AF = mybir.ActivationFunctionType

    nc = bacc.Bacc("TRN2", target_bir_lowering=False, debug=False,
                   num_devices=N_CORES)

    xT = nc.dram_tensor("xt", [HIDDEN, SEQ], fp16, kind="ExternalInput").ap()
    wqT = nc.dram_tensor("wqt", [HIDDEN, GDIM], fp16, kind="ExternalInput").ap()
    wkT = nc.dram_tensor("wkt", [HIDDEN, GDIM], fp16, kind="ExternalInput").ap()
    wvT = nc.dram_tensor("wvt", [HIDDEN, GDIM], fp16, kind="ExternalInput").ap()
    woT = nc.dram_tensor("wot", [GDIM, HIDDEN], fp16, kind="ExternalInput").ap()
    bq = nc.dram_tensor("bq", [GDIM], fp32, kind="ExternalInput").ap()
    bk = nc.dram_tensor("bk", [GDIM], fp32, kind="ExternalInput").ap()
    bv = nc.dram_tensor("bv", [GDIM], fp16, kind="ExternalInput").ap()
    out = nc.dram_tensor("out", [SEQ, HIDDEN], fp32, kind="ExternalOutput").ap()

    xT3 = xT.rearrange("(c p) s -> c p s", p=128)
    wq3 = wqT.rearrange("(c p) d -> c p d", p=128)
    wk3 = wkT.rearrange("(c p) d -> c p d", p=128)
    wv3 = wvT.rearrange("(c p) d -> c p d", p=128)
    wo3 = woT.rearrange("(c p) d -> c p d", p=128)
    out3 = out.rearrange("(t p) c -> t p c", p=128)

    with tile.TileContext(nc) as tc:
        with (
            tc.tile_pool(name="big", bufs=6) as big,     # 2MB slots
            tc.tile_pool(name="res", bufs=1) as res,
            tc.tile_pool(name="epp", bufs=2) as epp,
            tc.tile_pool(name="small", bufs=1) as small,
            tc.tile_pool(name="rec", bufs=2) as rec,
            tc.tile_pool(name="outp", bufs=3) as outp,
            tc.tile_pool(name="ps_a", bufs=2, space=bass.MemorySpace.PSUM) as ps_a,
            tc.tile_pool(name="ps_sc", bufs=2, space=bass.MemorySpace.PSUM) as ps_sc,
            tc.tile_pool(name="ps_sum", bufs=2, space=bass.MemorySpace.PSUM) as ps_sum,
            tc.tile_pool(name="ps_ctx", bufs=2, space=bass.MemorySpace.PSUM) as ps_ctx,
        ):
            wq_sb = big.tile([128, KC * GDIM], fp16, tag="big")
            wk_sb = big.tile([128, KC * GDIM], fp16, tag="big")
            wv_sb = big.tile([128, KC * GDIM], fp16, tag="big")

            qT_sb = res.tile([128, GHEADS * SEQ], fp16, tag="qT")
            kT_sb = res.tile([128, GHEADS * SEQ], fp16, tag="kT")
            v_sb = res.tile([128, MT * GDIM], fp16, tag="v")
            ctx_sb = res.tile([128, GHEADS * SEQ], fp16, tag="ctx")

            bq_sb = small.tile([128, GHEADS], fp32, tag="bq")
            bk_sb = small.tile([128, GHEADS], fp32, tag="bk")
            bv_sb = small.tile([1, GDIM], fp16, tag="bv")
            ones_sb = small.tile([128, 128], fp16, tag="ones")
            onesrow = small.tile([1, 128], fp16, tag="onesrow")
            nc.gpsimd.memset(ones_sb[:], 1.0)
            nc.gpsimd.memset(onesrow[:1, :], 1.0)

            # ---------- P0: initial DMAs + HAM warmup ----------
            xv = [big.tile([128, KC * 512], fp16, tag="big", name=f"xv{s}")
                  for s in range(SB)]
            for c in range(KC):
                nc.sync.dma_start(wv_sb[:, c * GDIM:(c + 1) * GDIM], wv3[c])
                nc.sync.dma_start(xv[0][:, c * 512:(c + 1) * 512],
                                  xT3[c, :, 0:512])
            nc.sync.dma_start(bv_sb[:1, :], bv.unsqueeze(0))
            nc.sync.dma_start(bq_sb[:], bq.rearrange("(m p) -> p m", p=128))
            nc.sync.dma_start(bk_sb[:], bk.rearrange("(m p) -> p m", p=128))

            warm = ps_a.tile([128, 128], fp32, tag="ps_a", name="warm")
            for _ in range(48):
                nc.tensor.matmul(warm[:], ones_sb[:], ones_sb[:],
                                 start=True, stop=True)

            def warm_trickle(n):
                wt = ps_ctx.tile([128, 64], fp32, tag="ps_ctx", name="wt")
                for _ in range(n):
                    nc.tensor.matmul(wt[:], ones_sb[:], ones_sb[:, :64],
                                     start=True, stop=True)

            # ---------- helpers ----------
            def qk_tile(xblk, w_sb, b_sb, dst, m, s0):
                """One [128 dims, 512 seq] q/k projection tile + bias copy."""
                ps = ps_a.tile([128, 512], fp32, tag="ps_a", name="psqk")
                for c in range(KC):
                    nc.tensor.matmul(
                        ps[:],
                        w_sb[:, c * GDIM + m * 128: c * GDIM + (m + 1) * 128],
                        xblk[:, c * 512:(c + 1) * 512],
                        start=(c == 0), stop=(c == KC - 1))
                nc.scalar.activation(
                    dst[:, m * SEQ + s0: m * SEQ + s0 + 512],
                    ps[:], AF.Identity, bias=b_sb[:, m:m + 1])

            def v_tile(xblk, sb, t):
                st = sb * 4 + t
                ps = ps_a.tile([128, 512], fp32, tag="ps_a")
                for c in range(KC):
                    nc.tensor.matmul(
                        ps[:],
                        xblk[:, c * 512 + t * 128: c * 512 + (t + 1) * 128],
                        wv_sb[:, c * GDIM:(c + 1) * GDIM],
                        start=(c == 0), stop=False)
                nc.tensor.matmul(ps[:], onesrow[:1, :], bv_sb[:1, :],
                                 start=False, stop=True)
                nc.vector.tensor_copy(v_sb[:, st * GDIM:(st + 1) * GDIM], ps[:])

            # P1 only consumes head 0's slice of wq/wk (128 of 512 cols per
            # chunk) — load just those up front; the rest streams during P1
            # and is only needed once the P2 filler starts.
            for c in range(KC):
                nc.sync.dma_start(wq_sb[:, c * GDIM: c * GDIM + 128],
                                  wq3[c][:, 0:128])
                nc.sync.dma_start(wk_sb[:, c * GDIM: c * GDIM + 128],
                                  wk3[c][:, 0:128])

            # ---------- P1: v projection + head-0 q/k projection ----------
            for sb in range(SB):
                s0 = sb * 512
                if sb + 1 < SB:
                    for c in range(KC):
                        nc.sync.dma_start(
                            xv[sb + 1][:, c * 512:(c + 1) * 512],
                            xT3[c, :, s0 + 512:s0 + 1024])
                if sb == 2:  # rest of wq/wk behind the x prefetches
                    for c in range(KC):
                        nc.sync.dma_start(
                            wq_sb[:, c * GDIM + 128:(c + 1) * GDIM],
                            wq3[c][:, 128:GDIM])
                        nc.sync.dma_start(
                            wk_sb[:, c * GDIM + 128:(c + 1) * GDIM],
                            wk3[c][:, 128:GDIM])
                for t in range(4):
                    v_tile(xv[sb], sb, t)
                qk_tile(xv[sb], wq_sb, bq_sb, qT_sb, 0, s0)
                qk_tile(xv[sb], wk_sb, bk_sb, kT_sb, 0, s0)

            # ---------- P2/P3: attention windows with PE filler ----------
            state = {}
            pend = []

            def drain(bi, kp):
                h, qb, eblk, ep, sums, ctxp = state[bi]
                for kc in (2 * kp, 2 * kp + 1):
                    nc.tensor.matmul(ctxp[:],
                                     v_sb[:, kc * GDIM + h * 128:
                                          kc * GDIM + (h + 1) * 128],
                                     eblk[:, kc * 512:(kc + 1) * 512],
                                     start=(kc == 0), stop=(kc == KC - 1))
                if kp == KC // 2 - 1:
                    nc.tensor.matmul(sums[:], ones_sb[:],
                                     ep[:, 0:512], start=True, stop=True)
                    finish(bi)

            def finish(bi):
                h, qb, eblk, ep, sums, ctxp = state.pop(bi)
                q0 = qb * 512
                recip = rec.tile([128, 512], fp32, tag="recip")
                nc.vector.reciprocal_approx_fast(recip[:], sums[:])
                nc.vector.tensor_mul(ctx_sb[:, h * SEQ + q0: h * SEQ + q0 + 512],
                                     ctxp[:], recip[:])

            # filler generators -------------------------------------------
            def proj_filler(h):
                """Yield 128 single-MM closures projecting head h's qT/kT,
                with x re-streamed per seq block (2 big-pool slots cycle)."""
                xb = {}

                def load_x(sb):
                    t = big.tile([128, KC * 512], fp16, tag="big",
                                 name=f"xh{h}_{sb}")
                    s0 = sb * 512
                    for c in range(KC):
                        nc.sync.dma_start(t[:, c * 512:(c + 1) * 512],
                                          xT3[c, :, s0:s0 + 512])
                    return t

                xb[0] = load_x(0)
                for sb in range(SB):
                    if sb + 1 < SB:
                        xb[sb + 1] = load_x(sb + 1)
                    s0 = sb * 512
                    for w_sb, b_sb, dst, nm in ((wq_sb, bq_sb, qT_sb, "q"),
                                                (wk_sb, bk_sb, kT_sb, "k")):
                        ps = ps_a.tile([128, 512], fp32, tag="ps_a",
                                       name=f"p{nm}{h}_{sb}")
                        for c in range(KC):
                            def mm(c=c, ps=ps, w_sb=w_sb, b_sb=b_sb, dst=dst,
                                   sb=sb, s0=s0):
                                nc.tensor.matmul(
                                    ps[:],
                                    w_sb[:, c * GDIM + h * 128:
                                         c * GDIM + (h + 1) * 128],
                                    xb[sb][:, c * 512:(c + 1) * 512],
                                    start=(c == 0), stop=(c == KC - 1))
                                if c == KC - 1:
                                    nc.scalar.activation(
                                        dst[:, h * SEQ + s0: h * SEQ + s0 + 512],
                                        ps[:], AF.Identity,
                                        bias=b_sb[:, h:h + 1])
                            yield mm

            def oproj_filler(qb, pools=None):
                """Yield 64 single-MM closures for o_proj q-tiles of block qb
                (all heads' ctx for qb must be finished)."""
                pools = pools or [(ps_a, "ps_a")]
                for i, (mq, oc) in enumerate(
                        (mq, oc) for mq in range(qb * 4, qb * 4 + 4)
                        for oc in range(4)):
                        pool, ptag = pools[i % len(pools)]
                        ps = pool.tile([128, 512], fp32, tag=ptag,
                                       name=f"po{mq}_{oc}")
                        for hh in range(GHEADS):
                            def mm(ps=ps, hh=hh, mq=mq, oc=oc, i=i):
                                nc.tensor.matmul(
                                    ps[:],
                                    ctx_sb[:, hh * SEQ + mq * 128:
                                           hh * SEQ + (mq + 1) * 128],
                                    wo_sb[:, hh * HIDDEN + oc * 512:
                                          hh * HIDDEN + (oc + 1) * 512],
                                    start=(hh == 0), stop=(hh == GHEADS - 1))
                                if hh == GHEADS - 1:
                                    o_t = outp.tile([128, 512], fp32, tag="out")
                                    if i % 2 == 0:
                                        nc.vector.tensor_copy(o_t[:], ps[:])
                                    else:
                                        nc.scalar.activation(o_t[:], ps[:],
                                                             AF.Copy)
                                    nc.sync.dma_start(
                                        out3[mq, :, oc * 512:(oc + 1) * 512],
                                        o_t[:])
                            yield mm

            bi = 0
            for h in range(GHEADS):
                if h < GHEADS - 1:
                    filler = proj_filler(h + 1)
                    per_stage = 4
                else:
                    wo_sb = big.tile([128, GHEADS * HIDDEN], fp16, tag="big")
                    for c in range(GHEADS):
                        nc.sync.dma_start(
                            wo_sb[:, c * HIDDEN:(c + 1) * HIDDEN], wo3[c])
                    filler = None  # switched per q-block below
                    per_stage = 8
                for qb in range(QB):
                    if h == GHEADS - 1 and qb >= 1:
                        filler = oproj_filler(qb - 1)
                    hq = h * SEQ
                    q0 = qb * 512
                    eblk = big.tile([128, KC * 512], fp16, tag="big",
                                    name="eblk")
                    ep = epp.tile([128, KC // 2 * 512], fp16, tag="ep")
                    sums = ps_sum.tile([128, 512], fp32, tag="ps_sum")
                    ctxp = ps_ctx.tile([128, 512], fp32, tag="ps_ctx")
                    state[bi] = (h, qb, eblk, ep, sums, ctxp)
                    for kp in range(KC // 2):
                        for i in (0, 1):
                            kc = 2 * kp + i
                            sc = ps_sc.tile([128, 512], fp32, tag="ps_sc")
                            nc.tensor.matmul(
                                sc[:],
                                kT_sb[:, hq + kc * 128: hq + (kc + 1) * 128],
                                qT_sb[:, hq + q0: hq + q0 + 512],
                                start=True, stop=True)
                            nc.scalar.activation(
                                eblk[:, kc * 512:(kc + 1) * 512], sc[:], AF.Exp)
                        nc.vector.tensor_add(
                            ep[:, kp * 512:(kp + 1) * 512],
                            eblk[:, (2 * kp) * 512:(2 * kp + 1) * 512],
                            eblk[:, (2 * kp + 1) * 512:(2 * kp + 2) * 512])
                        if kp % 2 == 1:
                            nc.vector.tensor_add(
                                ep[:, (kp - 1) * 512: kp * 512],
                                ep[:, (kp - 1) * 512: kp * 512],
                                ep[:, kp * 512:(kp + 1) * 512])
                        if kp % 4 == 3:
                            nc.vector.tensor_add(
                                ep[:, (kp - 3) * 512:(kp - 2) * 512],
                                ep[:, (kp - 3) * 512:(kp - 2) * 512],
                                ep[:, (kp - 1) * 512: kp * 512])
                        if kp == KC // 2 - 1:
                            nc.vector.tensor_add(
                                ep[:, 0:512], ep[:, 0:512],
                                ep[:, 4 * 512:5 * 512])
                        for b_kp in pend:
                            drain(*b_kp)
                        pend = [(bi, kp)]
                        if filler is not None:
                            for _ in range(per_stage):
                                mm = next(filler, None)
                                if mm is not None:
                                    mm()
                    if h == GHEADS - 1 and filler is not None:
                        for mm in filler:  # defensive: never drop filler work
                            mm()
                    bi += 1
                if h < GHEADS - 1 and filler is not None:
                    for mm in filler:
                        mm()
            for b_kp in pend:
                drain(*b_kp)

            # ---------- P4: leftover o_proj (last q-block) ----------
            # attention PSUM pools are idle now; rotate across them so the
            # PE never waits on a copy to release a bank
            for mm in oproj_filler(QB - 1, pools=[(ps_a, "ps_a"),
                                                  (ps_sum, "ps_sum"),
                                                  (ps_ctx, "ps_ctx")]):
                mm()

    nc.compile()
    return nc


def kernel(x, wq, bq, wk, bk, wv, bv, wo, bo):
    from concourse import bass_utils

    if "nc" not in _CACHE:
        _CACHE["nc"] = _build()
    nc = _CACHE["nc"]

    x = np.asarray(x, np.float32)
    scale = np.float32(1.0 / np.sqrt(HEAD_DIM))

    xT = [np.ascontiguousarray(x[b].T).astype(np.float16) for b in range(BATCH)]
    in_maps = []
    for j in range(N_CORES):
        b, g = divmod(j, GROUPS)
        ds = slice(g * GDIM, (g + 1) * GDIM)
        in_maps.append({
            "xt": xT[b],
            "wqt": np.ascontiguousarray((np.asarray(wq)[ds] * scale).T).astype(np.float16),
            "wkt": np.ascontiguousarray(np.asarray(wk)[ds].T).astype(np.float16),
            "wvt": np.ascontiguousarray(np.asarray(wv)[ds].T).astype(np.float16),
            "wot": np.ascontiguousarray(np.asarray(wo)[:, ds].T).astype(np.float16),
            "bq": (np.asarray(bq)[ds] * scale).astype(np.float32),
            "bk": np.asarray(bk)[ds].astype(np.float32),
            "bv": np.asarray(bv)[ds].astype(np.float16),
        })

    res = bass_utils.run_bass_kernel_spmd(
        nc, in_maps, core_ids=list(range(N_CORES)),
        **_CACHE.get("run_kwargs", {}))
    _CACHE["last_res"] = res

    out = np.zeros((BATCH, SEQ, HIDDEN), np.float32)
    for j in range(N_CORES):
        b = j // GROUPS
        out[b] += res.results[j]["out"]
    out += np.asarray(bo, np.float32)
    return out


# revision 24
# speedup vs baseline: 1.0186x; 1.0186x over previous
"""Multi-head attention (B=2, S=2048, H=2048, 16 heads, d=128) on 8 TRN2
NeuronCores.

Sharding: 2-way batch x 4-way head-group tensor parallel. Core j handles
batch j//4 and heads 4*(j%4)..4*(j%4)+3 (a 512-wide slice of the qkv
projection output dim / o_proj input dim). Each core returns a partial
o_proj output [S, H]; the host sums the 4 partials per batch and adds bo.

On-device compute per core (fp16 matmul operands, fp32 PSUM accumulation),
arranged as one interleaved PE stream so the tensor engine never waits on
the ACT-bound attention inner loop:

  P1: v = xT.T @ wvT (+bias) and head 0's qT/kT projections, seq-blocked.
  P2: for h in 0..2: attention(h) stages with head h+1's qT/kT projection
      matmuls as per-stage PE filler (x re-streamed from HBM per head).
  P3: attention(h=3) with o_proj matmuls of completed q-blocks as filler.
  P4: leftover o_proj + output staging.

Attention per (head, q-block), scoresT layout [k, q]:
  scoresT = kT_h.T @ qT_h -> exp on ScalarE -> expT (fp16)
  VectorE pair-adds exp chunks; sums += ones128.T @ pair  (half-cost rowsum)
  ctxT += v_chunk.T @ expT ; ctxT_norm = ctxT * approx_recip(sums)
"""
import sys

if "/opt/trn_rl_repo" not in sys.path:
    sys.path.insert(0, "/opt/trn_rl_repo")

import numpy as np

HIDDEN = 2048
HEADS = 16
HEAD_DIM = 128
BATCH = 2
SEQ = 2048

N_CORES = 8
GROUPS = 4               # head groups (cores per batch)
GDIM = HIDDEN // GROUPS  # 512 dims per core
GHEADS = GDIM // HEAD_DIM  # 4 heads per core
KC = HIDDEN // 128       # 16 contraction chunks
SB = 4                   # seq blocks of 512
QB = SEQ // 512          # 4 q-blocks in attention
MT = SEQ // 128          # 16 seq tiles of 128
NST = QB * KC // 2       # 32 attention stages per head

_CACHE = {}


def _build():
    import concourse.bacc as bacc
    import concourse.bass as bass
    import concourse.mybir as mybir
    import concourse.tile as tile

    fp16 = mybir.dt.float16
    fp32 = mybir.dt.float32
    AF = mybir.ActivationFunctionType

    nc = bacc.Bacc("TRN2", target_bir_lowering=False, debug=False,
                   num_devices=N_CORES)

    xT = nc.dram_tensor("xt", [HIDDEN, SEQ], fp16, kind="ExternalInput").ap()
    wqT = nc.dram_tensor("wqt", [HIDDEN, GDIM], fp16, kind="ExternalInput").ap()
    wkT = nc.dram_tensor("wkt", [HIDDEN, GDIM], fp16, kind="ExternalInput").ap()
    wvT = nc.dram_tensor("wvt", [HIDDEN, GDIM], fp16, kind="ExternalInput").ap()
    woT = nc.dram_tensor("wot", [GDIM, HIDDEN], fp16, kind="ExternalInput").ap()
    bq = nc.dram_tensor("bq", [GDIM], fp32, kind="ExternalInput").ap()
    bk = nc.dram_tensor("bk", [GDIM], fp32, kind="ExternalInput").ap()
    bv = nc.dram_tensor("bv", [GDIM], fp16, kind="ExternalInput").ap()
    out = nc.dram_tensor("out", [SEQ, HIDDEN], fp32, kind="ExternalOutput").ap()

    xT3 = xT.rearrange("(c p) s -> c p s", p=128)
    wq3 = wqT.rearrange("(c p) d -> c p d", p=128)
    wk3 = wkT.rearrange("(c p) d -> c p d", p=128)
    wv3 = wvT.rearrange("(c p) d -> c p d", p=128)
    wo3 = woT.rearrange("(c p) d -> c p d", p=128)
    out3 = out.rearrange("(t p) c -> t p c", p=128)

    with tile.TileContext(nc) as tc:
        with (
            tc.tile_pool(name="big", bufs=6) as big,     # 2MB slots
            tc.tile_pool(name="res", bufs=1) as res,
            tc.tile_pool(name="epp", bufs=2) as epp,
            tc.tile_pool(name="small", bufs=1) as small,
            tc.tile_pool(name="rec", bufs=2) as rec,
            tc.tile_pool(name="outp", bufs=3) as outp,
            tc.tile_pool(name="ps_a", bufs=2, space=bass.MemorySpace.PSUM) as ps_a,
            tc.tile_pool(name="ps_sc", bufs=2, space=bass.MemorySpace.PSUM) as ps_sc,
            tc.tile_pool(name="ps_sum", bufs=2, space=bass.MemorySpace.PSUM) as ps_sum,
            tc.tile_pool(name="ps_ctx", bufs=2, space=bass.MemorySpace.PSUM) as ps_ctx,
        ):
            wq_sb = big.tile([128, KC * GDIM], fp16, tag="big")
            wk_sb = big.tile([128, KC * GDIM], fp16, tag="big")
            wv_sb = big.tile([128, KC * GDIM], fp16, tag="big")

            qT_sb = res.tile([128, GHEADS * SEQ], fp16, tag="qT")
            kT_sb = res.tile([128, GHEADS * SEQ], fp16, tag="kT")
            v_sb = res.tile([128, MT * GDIM], fp16, tag="v")
            ctx_sb = res.tile([128, GHEADS * SEQ], fp16, tag="ctx")

            bq_sb = small.tile([128, GHEADS], fp32, tag="bq")
            bk_sb = small.tile([128, GHEADS], fp32, tag="bk")
            bv_sb = small.tile([1, GDIM], fp16, tag="bv")
            ones_sb = small.tile([128, 128], fp16, tag="ones")
            onesrow = small.tile([1, 128], fp16, tag="onesrow")
            nc.gpsimd.memset(ones_sb[:], 1.0)
            nc.gpsimd.memset(onesrow[:1, :], 1.0)

            # ---------- P0: initial DMAs + HAM warmup ----------
            xv = [big.tile([128, KC * 512], fp16, tag="big", name=f"xv{s}")
                  for s in range(SB)]
            for c in range(KC):
                nc.sync.dma_start(wv_sb[:, c * GDIM:(c + 1) * GDIM], wv3[c])
                nc.sync.dma_start(xv[0][:, c * 512:(c + 1) * 512],
                                  xT3[c, :, 0:512])
            nc.sync.dma_start(bv_sb[:1, :], bv.unsqueeze(0))
            nc.sync.dma_start(bq_sb[:], bq.rearrange("(m p) -> p m", p=128))
            nc.sync.dma_start(bk_sb[:], bk.rearrange("(m p) -> p m", p=128))

            warm = ps_a.tile([128, 128], fp32, tag="ps_a", name="warm")
            for _ in range(48):
                nc.tensor.matmul(warm[:], ones_sb[:], ones_sb[:],
                                 start=True, stop=True)

            def warm_trickle(n):
                wt = ps_ctx.tile([128, 64], fp32, tag="ps_ctx", name="wt")
                for _ in range(n):
                    nc.tensor.matmul(wt[:], ones_sb[:], ones_sb[:, :64],
                                     start=True, stop=True)

            # ---------- helpers ----------
            def qk_tile(xblk, w_sb, b_sb, dst, m, s0):
                """One [128 dims, 512 seq] q/k projection tile + bias copy."""
                ps = ps_a.tile([128, 512], fp32, tag="ps_a", name="psqk")
                for c in range(KC):
                    nc.tensor.matmul(
                        ps[:],
                        w_sb[:, c * GDIM + m * 128: c * GDIM + (m + 1) * 128],
                        xblk[:, c * 512:(c + 1) * 512],
                        start=(c == 0), stop=(c == KC - 1))
                nc.scalar.activation(
                    dst[:, m * SEQ + s0: m * SEQ + s0 + 512],
                    ps[:], AF.Identity, bias=b_sb[:, m:m + 1])

            def v_tile(xblk, sb, t):
                st = sb * 4 + t
                ps = ps_a.tile([128, 512], fp32, tag="ps_a")
                for c in range(KC):
                    nc.tensor.matmul(
                        ps[:],
                        xblk[:, c * 512 + t * 128: c * 512 + (t + 1) * 128],
                        wv_sb[:, c * GDIM:(c + 1) * GDIM],
                        start=(c == 0), stop=False)
                nc.tensor.matmul(ps[:], onesrow[:1, :], bv_sb[:1, :],
                                 start=False, stop=True)
                nc.vector.tensor_copy(v_sb[:, st * GDIM:(st + 1) * GDIM], ps[:])

            # P1 only consumes head 0's slice of wq/wk (128 of 512 cols per
            # chunk) — load just those up front; the rest streams during P1
            # and is only needed once the P2 filler starts.
            for c in range(KC):
                nc.sync.dma_start(wq_sb[:, c * GDIM: c * GDIM + 128],
                                  wq3[c][:, 0:128])
                nc.sync.dma_start(wk_sb[:, c * GDIM: c * GDIM + 128],
                                  wk3[c][:, 0:128])

            # ---------- P1: v projection + head-0 q/k projection ----------
            for sb in range(SB):
                s0 = sb * 512
                if sb + 1 < SB:
                    for c in range(KC):
                        nc.sync.dma_start(
                            xv[sb + 1][:, c * 512:(c + 1) * 512],
                            xT3[c, :, s0 + 512:s0 + 1024])
                if sb == 2:  # rest of wq/wk behind the x prefetches
                    for c in range(KC):
                        nc.sync.dma_start(
                            wq_sb[:, c * GDIM + 128:(c + 1) * GDIM],
                            wq3[c][:, 128:GDIM])
                        nc.sync.dma_start(
                            wk_sb[:, c * GDIM + 128:(c + 1) * GDIM],
                            wk3[c][:, 128:GDIM])
                for t in range(4):
                    v_tile(xv[sb], sb, t)
                    if sb == 0:
                        warm_trickle(6)
                qk_tile(xv[sb], wq_sb, bq_sb, qT_sb, 0, s0)
                qk_tile(xv[sb], wk_sb, bk_sb, kT_sb, 0, s0)

            # ---------- P2/P3: attention windows with PE filler ----------
            state = {}
            pend = []

            def drain(bi, kp):
                h, qb, eblk, ep, sums, ctxp = state[bi]
                for kc in (2 * kp, 2 * kp + 1):
                    nc.tensor.matmul(ctxp[:],
                                     v_sb[:, kc * GDIM + h * 128:
                                          kc * GDIM + (h + 1) * 128],
                                     eblk[:, kc * 512:(kc + 1) * 512],
                                     start=(kc == 0), stop=(kc == KC - 1))
                if kp == KC // 2 - 1:
                    nc.tensor.matmul(sums[:], ones_sb[:],
                                     ep[:, 0:512], start=True, stop=True)
                    finish(bi)

            def finish(bi):
                h, qb, eblk, ep, sums, ctxp = state.pop(bi)
                q0 = qb * 512
                recip = rec.tile([128, 512], fp32, tag="recip")
                nc.vector.reciprocal_approx_fast(recip[:], sums[:])
                nc.vector.tensor_mul(ctx_sb[:, h * SEQ + q0: h * SEQ + q0 + 512],
                                     ctxp[:], recip[:])

            # filler generators -------------------------------------------
            def proj_filler(h):
                """Yield 128 single-MM closures projecting head h's qT/kT,
                with x re-streamed per seq block (2 big-pool slots cycle)."""
                xb = {}

                def load_x(sb):
                    t = big.tile([128, KC * 512], fp16, tag="big",
                                 name=f"xh{h}_{sb}")
                    s0 = sb * 512
                    for c in range(KC):
                        nc.sync.dma_start(t[:, c * 512:(c + 1) * 512],
                                          xT3[c, :, s0:s0 + 512])
                    return t

                xb[0] = load_x(0)
                for sb in range(SB):
                    if sb + 1 < SB:
                        xb[sb + 1] = load_x(sb + 1)
                    s0 = sb * 512
                    for w_sb, b_sb, dst, nm in ((wq_sb, bq_sb, qT_sb, "q"),
                                                (wk_sb, bk_sb, kT_sb, "k")):
                        ps = ps_a.tile([128, 512], fp32, tag="ps_a",
                                       name=f"p{nm}{h}_{sb}")
                        for c in range(KC):
                            def mm(c=c, ps=ps, w_sb=w_sb, b_sb=b_sb, dst=dst,
                                   sb=sb, s0=s0):
                                nc.tensor.matmul(
                                    ps[:],
                                    w_sb[:, c * GDIM + h * 128:
                                         c * GDIM + (h + 1) * 128],
                                    xb[sb][:, c * 512:(c + 1) * 512],
                                    start=(c == 0), stop=(c == KC - 1))
                                if c == KC - 1:
                                    nc.scalar.activation(
                                        dst[:, h * SEQ + s0: h * SEQ + s0 + 512],
                                        ps[:], AF.Identity,
                                        bias=b_sb[:, h:h + 1])
                            yield mm

            def oproj_filler(qb, pools=None):
                """Yield 64 single-MM closures for o_proj q-tiles of block qb
                (all heads' ctx for qb must be finished)."""
                pools = pools or [(ps_a, "ps_a")]
                for i, (mq, oc) in enumerate(
                        (mq, oc) for mq in range(qb * 4, qb * 4 + 4)
                        for oc in range(4)):
                        pool, ptag = pools[i % len(pools)]
                        ps = pool.tile([128, 512], fp32, tag=ptag,
                                       name=f"po{mq}_{oc}")
                        for hh in range(GHEADS):
                            def mm(ps=ps, hh=hh, mq=mq, oc=oc, i=i):
                                nc.tensor.matmul(
                                    ps[:],
                                    ctx_sb[:, hh * SEQ + mq * 128:
                                           hh * SEQ + (mq + 1) * 128],
                                    wo_sb[:, hh * HIDDEN + oc * 512:
                                          hh * HIDDEN + (oc + 1) * 512],
                                    start=(hh == 0), stop=(hh == GHEADS - 1))
                                if hh == GHEADS - 1:
                                    o_t = outp.tile([128, 512], fp32, tag="out")
                                    if i % 2 == 0:
                                        nc.vector.tensor_copy(o_t[:], ps[:])
                                    else:
                                        nc.scalar.activation(o_t[:], ps[:],
                                                             AF.Copy)
                                    nc.sync.dma_start(
                                        out3[mq, :, oc * 512:(oc + 1) * 512],
                                        o_t[:])
                            yield mm

            bi = 0
            for h in range(GHEADS):
                if h < GHEADS - 1:
                    filler = proj_filler(h + 1)
                    per_stage = 4
                else:
                    wo_sb = big.tile([128, GHEADS * HIDDEN], fp16, tag="big")
                    for c in range(GHEADS):
                        nc.sync.dma_start(
                            wo_sb[:, c * HIDDEN:(c + 1) * HIDDEN], wo3[c])
                    filler = None  # switched per q-block below
                    per_stage = 8
                for qb in range(QB):
                    if h == GHEADS - 1 and qb >= 1:
                        filler = oproj_filler(qb - 1)
                    hq = h * SEQ
                    q0 = qb * 512
                    eblk = big.tile([128, KC * 512], fp16, tag="big",
                                    name="eblk")
                    ep = epp.tile([128, KC // 2 * 512], fp16, tag="ep")
                    sums = ps_sum.tile([128, 512], fp32, tag="ps_sum")
                    ctxp = ps_ctx.tile([128, 512], fp32, tag="ps_ctx")
                    state[bi] = (h, qb, eblk, ep, sums, ctxp)
                    for kp in range(KC // 2):
                        for i in (0, 1):
                            kc = 2 * kp + i
                            sc = ps_sc.tile([128, 512], fp32, tag="ps_sc")
                            nc.tensor.matmul(
                                sc[:],
                                kT_sb[:, hq + kc * 128: hq + (kc + 1) * 128],
                                qT_sb[:, hq + q0: hq + q0 + 512],
                                start=True, stop=True)
                            nc.scalar.activation(
                                eblk[:, kc * 512:(kc + 1) * 512], sc[:], AF.Exp)
                        nc.vector.tensor_add(
                            ep[:, kp * 512:(kp + 1) * 512],
                            eblk[:, (2 * kp) * 512:(2 * kp + 1) * 512],
                            eblk[:, (2 * kp + 1) * 512:(2 * kp + 2) * 512])
                        if kp % 2 == 1:
                            nc.vector.tensor_add(
                                ep[:, (kp - 1) * 512: kp * 512],
                                ep[:, (kp - 1) * 512: kp * 512],
                                ep[:, kp * 512:(kp + 1) * 512])
                        if kp % 4 == 3:
                            nc.vector.tensor_add(
                                ep[:, (kp - 3) * 512:(kp - 2) * 512],
                                ep[:, (kp - 3) * 512:(kp - 2) * 512],
                                ep[:, (kp - 1) * 512: kp * 512])
                        if kp == KC // 2 - 1:
                            nc.vector.tensor_add(
                                ep[:, 0:512], ep[:, 0:512],
                                ep[:, 4 * 512:5 * 512])
                        for b_kp in pend:
                            drain(*b_kp)
                        pend = [(bi, kp)]
                        if filler is not None:
                            for _ in range(per_stage):
                                mm = next(filler, None)
                                if mm is not None:
                                    mm()
                    if h == GHEADS - 1 and filler is not None:
                        for mm in filler:  # defensive: never drop filler work
                            mm()
                    bi += 1
                if h < GHEADS - 1 and filler is not None:
                    for mm in filler:
                        mm()
            for b_kp in pend:
                drain(*b_kp)

            # ---------- P4: leftover o_proj (last q-block) ----------
            # attention PSUM pools are idle now; rotate across them so the
            # PE never waits on a copy to release a bank
            for mm in oproj_filler(QB - 1, pools=[(ps_a, "ps_a"),
                                                  (ps_sum, "ps_sum"),
                                                  (ps_ctx, "ps_ctx")]):
                mm()

    nc.compile()
    return nc


def kernel(x, wq, bq, wk, bk, wv, bv, wo, bo):
    from concourse import bass_utils

    if "nc" not in _CACHE:
        _CACHE["nc"] = _build()
    nc = _CACHE["nc"]

    x = np.asarray(x, np.float32)
    scale = np.float32(1.0 / np.sqrt(HEAD_DIM))

    xT = [np.ascontiguousarray(x[b].T).astype(np.float16) for b in range(BATCH)]
    in_maps = []
    for j in range(N_CORES):
        b, g = divmod(j, GROUPS)
        ds = slice(g * GDIM, (g + 1) * GDIM)
        in_maps.append({
            "xt": xT[b],
            "wqt": np.ascontiguousarray((np.asarray(wq)[ds] * scale).T).astype(np.float16),
            "wkt": np.ascontiguousarray(np.asarray(wk)[ds].T).astype(np.float16),
            "wvt": np.ascontiguousarray(np.asarray(wv)[ds].T).astype(np.float16),
            "wot": np.ascontiguousarray(np.asarray(wo)[:, ds].T).astype(np.float16),
            "bq": (np.asarray(bq)[ds] * scale).astype(np.float32),
            "bk": np.asarray(bk)[ds].astype(np.float32),
            "bv": np.asarray(bv)[ds].astype(np.float16),
        })

    res = bass_utils.run_bass_kernel_spmd(
        nc, in_maps, core_ids=list(range(N_CORES)),
        **_CACHE.get("run_kwargs", {}))
    _CACHE["last_res"] = res

    out = np.zeros((BATCH, SEQ, HIDDEN), np.float32)
    for j in range(N_CORES):
        b = j // GROUPS
        out[b] += res.results[j]["out"]
    out += np.asarray(bo, np.float32)
    return out


# revision 25
# speedup vs baseline: 1.0373x; 1.0184x over previous
"""Multi-head attention (B=2, S=2048, H=2048, 16 heads, d=128) on 8 TRN2
NeuronCores.

Sharding: 2-way batch x 4-way head-group tensor parallel. Core j handles
batch j//4 and heads 4*(j%4)..4*(j%4)+3 (a 512-wide slice of the qkv
projection output dim / o_proj input dim). Each core returns a partial
o_proj output [S, H]; the host sums the 4 partials per batch and adds bo.

On-device compute per core (fp16 matmul operands, fp32 PSUM accumulation),
arranged as one interleaved PE stream so the tensor engine never waits on
the ACT-bound attention inner loop:

  P1: v = xT.T @ wvT (+bias) and head 0's qT/kT projections, seq-blocked.
  P2: for h in 0..2: attention(h) stages with head h+1's qT/kT projection
      matmuls as per-stage PE filler (x re-streamed from HBM per head).
  P3: attention(h=3) with o_proj matmuls of completed q-blocks as filler.
  P4: leftover o_proj + output staging.

Attention per (head, q-block), scoresT layout [k, q]:
  scoresT = kT_h.T @ qT_h -> exp on ScalarE -> expT (fp16)
  VectorE pair-adds exp chunks; sums += ones128.T @ pair  (half-cost rowsum)
  ctxT += v_chunk.T @ expT ; ctxT_norm = ctxT * approx_recip(sums)
"""
import sys

if "/opt/trn_rl_repo" not in sys.path:
    sys.path.insert(0, "/opt/trn_rl_repo")

import numpy as np

HIDDEN = 2048
HEADS = 16
HEAD_DIM = 128
BATCH = 2
SEQ = 2048

N_CORES = 8
GROUPS = 4               # head groups (cores per batch)
GDIM = HIDDEN // GROUPS  # 512 dims per core
GHEADS = GDIM // HEAD_DIM  # 4 heads per core
KC = HIDDEN // 128       # 16 contraction chunks
SB = 4                   # seq blocks of 512
QB = SEQ // 512          # 4 q-blocks in attention
MT = SEQ // 128          # 16 seq tiles of 128
NST = QB * KC // 2       # 32 attention stages per head

_CACHE = {}


def _build():
    import concourse.bacc as bacc
    import concourse.bass as bass
    import concourse.mybir as mybir
    import concourse.tile as tile

    fp16 = mybir.dt.float16
    fp32 = mybir.dt.float32
    AF = mybir.ActivationFunctionType

    nc = bacc.Bacc("TRN2", target_bir_lowering=False, debug=False,
                   num_devices=N_CORES)

    xT = nc.dram_tensor("xt", [SB, KC, 128, 512], fp16, kind="ExternalInput").ap()
    wqT = nc.dram_tensor("wqt", [HIDDEN, GDIM], fp16, kind="ExternalInput").ap()
    wkT = nc.dram_tensor("wkt", [HIDDEN, GDIM], fp16, kind="ExternalInput").ap()
    wvT = nc.dram_tensor("wvt", [HIDDEN, GDIM], fp16, kind="ExternalInput").ap()
    woT = nc.dram_tensor("wot", [GDIM, HIDDEN], fp16, kind="ExternalInput").ap()
    bq = nc.dram_tensor("bq", [GDIM], fp32, kind="ExternalInput").ap()
    bk = nc.dram_tensor("bk", [GDIM], fp32, kind="ExternalInput").ap()
    bv = nc.dram_tensor("bv", [GDIM], fp16, kind="ExternalInput").ap()
    out = nc.dram_tensor("out", [MT, 4, 128, 512], fp32, kind="ExternalOutput").ap()

    wq3 = wqT.rearrange("(c p) d -> c p d", p=128)
    wk3 = wkT.rearrange("(c p) d -> c p d", p=128)
    wv3 = wvT.rearrange("(c p) d -> c p d", p=128)
    wo3 = woT.rearrange("(c p) d -> c p d", p=128)

    with tile.TileContext(nc) as tc:
        with (
            tc.tile_pool(name="big", bufs=6) as big,     # 2MB slots
            tc.tile_pool(name="res", bufs=1) as res,
            tc.tile_pool(name="epp", bufs=2) as epp,
            tc.tile_pool(name="small", bufs=1) as small,
            tc.tile_pool(name="rec", bufs=2) as rec,
            tc.tile_pool(name="outp", bufs=3) as outp,
            tc.tile_pool(name="ps_a", bufs=2, space=bass.MemorySpace.PSUM) as ps_a,
            tc.tile_pool(name="ps_sc", bufs=2, space=bass.MemorySpace.PSUM) as ps_sc,
            tc.tile_pool(name="ps_sum", bufs=2, space=bass.MemorySpace.PSUM) as ps_sum,
            tc.tile_pool(name="ps_ctx", bufs=2, space=bass.MemorySpace.PSUM) as ps_ctx,
        ):
            wq_sb = big.tile([128, KC * GDIM], fp16, tag="big")
            wk_sb = big.tile([128, KC * GDIM], fp16, tag="big")
            wv_sb = big.tile([128, KC * GDIM], fp16, tag="big")

            qT_sb = res.tile([128, GHEADS * SEQ], fp16, tag="qT")
            kT_sb = res.tile([128, GHEADS * SEQ], fp16, tag="kT")
            v_sb = res.tile([128, MT * GDIM], fp16, tag="v")
            ctx_sb = res.tile([128, GHEADS * SEQ], fp16, tag="ctx")

            bq_sb = small.tile([128, GHEADS], fp32, tag="bq")
            bk_sb = small.tile([128, GHEADS], fp32, tag="bk")
            bv_sb = small.tile([1, GDIM], fp16, tag="bv")
            ones_sb = small.tile([128, 128], fp16, tag="ones")
            onesrow = small.tile([1, 128], fp16, tag="onesrow")
            nc.gpsimd.memset(ones_sb[:], 1.0)
            nc.gpsimd.memset(onesrow[:1, :], 1.0)

            # ---------- P0: initial DMAs + HAM warmup ----------
            xv = [big.tile([128, KC * 512], fp16, tag="big", name=f"xv{s}")
                  for s in range(SB)]
            for c in range(KC):
                nc.sync.dma_start(wv_sb[:, c * GDIM:(c + 1) * GDIM], wv3[c])
                nc.sync.dma_start(xv[0][:, c * 512:(c + 1) * 512],
                                  xT[0, c])
            nc.sync.dma_start(bv_sb[:1, :], bv.unsqueeze(0))
            nc.sync.dma_start(bq_sb[:], bq.rearrange("(m p) -> p m", p=128))
            nc.sync.dma_start(bk_sb[:], bk.rearrange("(m p) -> p m", p=128))

            warm = ps_a.tile([128, 128], fp32, tag="ps_a", name="warm")
            for _ in range(48):
                nc.tensor.matmul(warm[:], ones_sb[:], ones_sb[:],
                                 start=True, stop=True)

            def warm_trickle(n):
                wt = ps_ctx.tile([128, 64], fp32, tag="ps_ctx", name="wt")
                for _ in range(n):
                    nc.tensor.matmul(wt[:], ones_sb[:], ones_sb[:, :64],
                                     start=True, stop=True)

            # ---------- helpers ----------
            def qk_tile(xblk, w_sb, b_sb, dst, m, s0):
                """One [128 dims, 512 seq] q/k projection tile + bias copy."""
                ps = ps_a.tile([128, 512], fp32, tag="ps_a", name="psqk")
                for c in range(KC):
                    nc.tensor.matmul(
                        ps[:],
                        w_sb[:, c * GDIM + m * 128: c * GDIM + (m + 1) * 128],
                        xblk[:, c * 512:(c + 1) * 512],
                        start=(c == 0), stop=(c == KC - 1))
                nc.scalar.activation(
                    dst[:, m * SEQ + s0: m * SEQ + s0 + 512],
                    ps[:], AF.Identity, bias=b_sb[:, m:m + 1])

            def v_tile(xblk, sb, t):
                st = sb * 4 + t
                ps = ps_a.tile([128, 512], fp32, tag="ps_a")
                for c in range(KC):
                    nc.tensor.matmul(
                        ps[:],
                        xblk[:, c * 512 + t * 128: c * 512 + (t + 1) * 128],
                        wv_sb[:, c * GDIM:(c + 1) * GDIM],
                        start=(c == 0), stop=False)
                nc.tensor.matmul(ps[:], onesrow[:1, :], bv_sb[:1, :],
                                 start=False, stop=True)
                nc.vector.tensor_copy(v_sb[:, st * GDIM:(st + 1) * GDIM], ps[:])

            # P1 only consumes head 0's slice of wq/wk (128 of 512 cols per
            # chunk) — load just those up front; the rest streams during P1
            # and is only needed once the P2 filler starts.
            for c in range(KC):
                nc.sync.dma_start(wq_sb[:, c * GDIM: c * GDIM + 128],
                                  wq3[c][:, 0:128])
                nc.sync.dma_start(wk_sb[:, c * GDIM: c * GDIM + 128],
                                  wk3[c][:, 0:128])

            # ---------- P1: v projection + head-0 q/k projection ----------
            for sb in range(SB):
                s0 = sb * 512
                if sb + 1 < SB:
                    for c in range(KC):
                        nc.sync.dma_start(
                            xv[sb + 1][:, c * 512:(c + 1) * 512],
                            xT[sb + 1, c])
                if sb == 2:  # rest of wq/wk behind the x prefetches
                    for c in range(KC):
                        nc.sync.dma_start(
                            wq_sb[:, c * GDIM + 128:(c + 1) * GDIM],
                            wq3[c][:, 128:GDIM])
                        nc.sync.dma_start(
                            wk_sb[:, c * GDIM + 128:(c + 1) * GDIM],
                            wk3[c][:, 128:GDIM])
                for t in range(4):
                    v_tile(xv[sb], sb, t)
                    if sb == 0:
                        warm_trickle(6)
                qk_tile(xv[sb], wq_sb, bq_sb, qT_sb, 0, s0)
                qk_tile(xv[sb], wk_sb, bk_sb, kT_sb, 0, s0)

            # ---------- P2/P3: attention windows with PE filler ----------
            state = {}
            pend = []

            def drain(bi, kp):
                h, qb, eblk, ep, sums, ctxp = state[bi]
                for kc in (2 * kp, 2 * kp + 1):
                    nc.tensor.matmul(ctxp[:],
                                     v_sb[:, kc * GDIM + h * 128:
                                          kc * GDIM + (h + 1) * 128],
                                     eblk[:, kc * 512:(kc + 1) * 512],
                                     start=(kc == 0), stop=(kc == KC - 1))
                if kp == KC // 2 - 1:
                    nc.tensor.matmul(sums[:], ones_sb[:],
                                     ep[:, 0:512], start=True, stop=True)
                    finish(bi)

            def finish(bi):
                h, qb, eblk, ep, sums, ctxp = state.pop(bi)
                q0 = qb * 512
                recip = rec.tile([128, 512], fp32, tag="recip")
                nc.vector.reciprocal_approx_fast(recip[:], sums[:])
                nc.vector.tensor_mul(ctx_sb[:, h * SEQ + q0: h * SEQ + q0 + 512],
                                     ctxp[:], recip[:])

            # filler generators -------------------------------------------
            def proj_filler(h):
                """Yield 128 single-MM closures projecting head h's qT/kT,
                with x re-streamed per seq block (2 big-pool slots cycle)."""
                xb = {}

                def load_x(sb):
                    t = big.tile([128, KC * 512], fp16, tag="big",
                                 name=f"xh{h}_{sb}")
                    for c in range(KC):
                        nc.sync.dma_start(t[:, c * 512:(c + 1) * 512],
                                          xT[sb, c])
                    return t

                xb[0] = load_x(0)
                for sb in range(SB):
                    if sb + 1 < SB:
                        xb[sb + 1] = load_x(sb + 1)
                    s0 = sb * 512
                    for w_sb, b_sb, dst, nm in ((wq_sb, bq_sb, qT_sb, "q"),
                                                (wk_sb, bk_sb, kT_sb, "k")):
                        ps = ps_a.tile([128, 512], fp32, tag="ps_a",
                                       name=f"p{nm}{h}_{sb}")
                        for c in range(KC):
                            def mm(c=c, ps=ps, w_sb=w_sb, b_sb=b_sb, dst=dst,
                                   sb=sb, s0=s0):
                                nc.tensor.matmul(
                                    ps[:],
                                    w_sb[:, c * GDIM + h * 128:
                                         c * GDIM + (h + 1) * 128],
                                    xb[sb][:, c * 512:(c + 1) * 512],
                                    start=(c == 0), stop=(c == KC - 1))
                                if c == KC - 1:
                                    nc.scalar.activation(
                                        dst[:, h * SEQ + s0: h * SEQ + s0 + 512],
                                        ps[:], AF.Identity,
                                        bias=b_sb[:, h:h + 1])
                            yield mm

            def oproj_filler(qb, pools=None):
                """Yield 64 single-MM closures for o_proj q-tiles of block qb
                (all heads' ctx for qb must be finished)."""
                pools = pools or [(ps_a, "ps_a")]
                for i, (mq, oc) in enumerate(
                        (mq, oc) for mq in range(qb * 4, qb * 4 + 4)
                        for oc in range(4)):
                        pool, ptag = pools[i % len(pools)]
                        ps = pool.tile([128, 512], fp32, tag=ptag,
                                       name=f"po{mq}_{oc}")
                        for hh in range(GHEADS):
                            def mm(ps=ps, hh=hh, mq=mq, oc=oc, i=i):
                                nc.tensor.matmul(
                                    ps[:],
                                    ctx_sb[:, hh * SEQ + mq * 128:
                                           hh * SEQ + (mq + 1) * 128],
                                    wo_sb[:, hh * HIDDEN + oc * 512:
                                          hh * HIDDEN + (oc + 1) * 512],
                                    start=(hh == 0), stop=(hh == GHEADS - 1))
                                if hh == GHEADS - 1:
                                    o_t = outp.tile([128, 512], fp32, tag="out")
                                    if i % 2 == 0:
                                        nc.vector.tensor_copy(o_t[:], ps[:])
                                    else:
                                        nc.scalar.activation(o_t[:], ps[:],
                                                             AF.Copy)
                                    nc.sync.dma_start(
                                        out[mq, oc], o_t[:])
                            yield mm

            bi = 0
            for h in range(GHEADS):
                if h < GHEADS - 1:
                    filler = proj_filler(h + 1)
                    per_stage = 4
                else:
                    wo_sb = big.tile([128, GHEADS * HIDDEN], fp16, tag="big")
                    for c in range(GHEADS):
                        nc.sync.dma_start(
                            wo_sb[:, c * HIDDEN:(c + 1) * HIDDEN], wo3[c])
                    filler = None  # switched per q-block below
                    per_stage = 8
                for qb in range(QB):
                    if h == GHEADS - 1 and qb >= 1:
                        filler = oproj_filler(qb - 1)
                    hq = h * SEQ
                    q0 = qb * 512
                    eblk = big.tile([128, KC * 512], fp16, tag="big",
                                    name="eblk")
                    ep = epp.tile([128, KC // 2 * 512], fp16, tag="ep")
                    sums = ps_sum.tile([128, 512], fp32, tag="ps_sum")
                    ctxp = ps_ctx.tile([128, 512], fp32, tag="ps_ctx")
                    state[bi] = (h, qb, eblk, ep, sums, ctxp)
                    for kp in range(KC // 2):
                        for i in (0, 1):
                            kc = 2 * kp + i
                            sc = ps_sc.tile([128, 512], fp32, tag="ps_sc")
                            nc.tensor.matmul(
                                sc[:],
                                kT_sb[:, hq + kc * 128: hq + (kc + 1) * 128],
                                qT_sb[:, hq + q0: hq + q0 + 512],
                                start=True, stop=True)
                            nc.scalar.activation(
                                eblk[:, kc * 512:(kc + 1) * 512], sc[:], AF.Exp)
                        nc.vector.tensor_add(
                            ep[:, kp * 512:(kp + 1) * 512],
                            eblk[:, (2 * kp) * 512:(2 * kp + 1) * 512],
                            eblk[:, (2 * kp + 1) * 512:(2 * kp + 2) * 512])
                        if kp % 2 == 1:
                            nc.vector.tensor_add(
                                ep[:, (kp - 1) * 512: kp * 512],
                                ep[:, (kp - 1) * 512: kp * 512],
                                ep[:, kp * 512:(kp + 1) * 512])
                        if kp % 4 == 3:
                            nc.vector.tensor_add(
                                ep[:, (kp - 3) * 512:(kp - 2) * 512],
                                ep[:, (kp - 3) * 512:(kp - 2) * 512],
                                ep[:, (kp - 1) * 512: kp * 512])
                        if kp == KC // 2 - 1:
                            nc.vector.tensor_add(
                                ep[:, 0:512], ep[:, 0:512],
                                ep[:, 4 * 512:5 * 512])
                        for b_kp in pend:
                            drain(*b_kp)
                        pend = [(bi, kp)]
                        if filler is not None:
                            for _ in range(per_stage):
                                mm = next(filler, None)
                                if mm is not None:
                                    mm()
                    if h == GHEADS - 1 and filler is not None:
                        for mm in filler:  # defensive: never drop filler work
                            mm()
                    bi += 1
                if h < GHEADS - 1 and filler is not None:
                    for mm in filler:
                        mm()
            for b_kp in pend:
                drain(*b_kp)

            # ---------- P4: leftover o_proj (last q-block) ----------
            # attention PSUM pools are idle now; rotate across them so the
            # PE never waits on a copy to release a bank
            for mm in oproj_filler(QB - 1, pools=[(ps_a, "ps_a"),
                                                  (ps_sum, "ps_sum"),
                                                  (ps_ctx, "ps_ctx")]):
                mm()

    nc.compile()
    return nc


def kernel(x, wq, bq, wk, bk, wv, bv, wo, bo):
    from concourse import bass_utils

    if "nc" not in _CACHE:
        _CACHE["nc"] = _build()
    nc = _CACHE["nc"]

    x = np.asarray(x, np.float32)
    scale = np.float32(1.0 / np.sqrt(HEAD_DIM))

    xT = [np.ascontiguousarray(
        x[b].T.reshape(KC, 128, SB, 512).transpose(2, 0, 1, 3)).astype(np.float16)
        for b in range(BATCH)]
    in_maps = []
    for j in range(N_CORES):
        b, g = divmod(j, GROUPS)
        ds = slice(g * GDIM, (g + 1) * GDIM)
        in_maps.append({
            "xt": xT[b],
            "wqt": np.ascontiguousarray((np.asarray(wq)[ds] * scale).T).astype(np.float16),
            "wkt": np.ascontiguousarray(np.asarray(wk)[ds].T).astype(np.float16),
            "wvt": np.ascontiguousarray(np.asarray(wv)[ds].T).astype(np.float16),
            "wot": np.ascontiguousarray(np.asarray(wo)[:, ds].T).astype(np.float16),
            "bq": (np.asarray(bq)[ds] * scale).astype(np.float32),
            "bk": np.asarray(bk)[ds].astype(np.float32),
            "bv": np.asarray(bv)[ds].astype(np.float16),
        })

    res = bass_utils.run_bass_kernel_spmd(
        nc, in_maps, core_ids=list(range(N_CORES)),
        **_CACHE.get("run_kwargs", {}))
    _CACHE["last_res"] = res

    out = np.zeros((BATCH, MT, 4, 128, 512), np.float32)
    for j in range(N_CORES):
        b = j // GROUPS
        out[b] += res.results[j]["out"]
    out = out.transpose(0, 1, 3, 2, 4).reshape(BATCH, SEQ, HIDDEN)
    out = out + np.asarray(bo, np.float32)
    return out
